# revision 23
# baseline (speedup 1.0000x reference)
"""Trainium2 Bass kernel for nn_LocalRegionLevelLoss (8-core data parallel).

loss = sum_{b,p,r} softmax_r(pos@img^T) * relu(margin + max_n(neg@img^T) - pos@img^T)

Strategy:
  - Pure data parallelism: batch dim (128) sharded 16-per-core across 8 cores.
  - fp8 E3M4 (float8e3) inputs: the kernel is HBM-bandwidth bound
    (45.6MB/core at fp32 vs a ~358GB/s/core HBM limit), and the loss is a
    2560-term softmax-weighted sum, so fp8 input quantization noise largely
    averages out (~7e-4 end-to-end rel err; e4m3 measured 1.3e-2).
  - The 20 positive rows are appended to the 640 negative rows of each
    batch's moving operand, so ONE stream of matmuls per batch produces
    both neg sims and (transposed) pos sims from the same stationary img
    chunks: no separate pos matmuls or extra weight loads.
  - HWDGE DMAs from one engine drain FIFO, so the negpos stream is issued
    in small-first groups (1,1,2,4,4,4 batches) to cut the pipeline-fill
    stall, while a burst of dummy ident matmuls pre-warms the PE HAM clock
    gate (cold PE = 1.2GHz, warm = 2.4GHz) during the first DMA.
  - Per batch only 3 cheap ops besides the matmuls: 2x reduce_max of the
    neg sims into a [36, 16*20] stack, 1 ScalarE copy of the transposed
    pos sims into a [36, 16*20] stack.  The softmax-weighted hinge runs 3x
    on [120, 36] PE-transposed chunks, interleaved into the batch loop as
    their source batches complete, so almost no serial tail remains.
  - Each core emits one f32 partial loss; host sums the 8 partials.
"""

import os
import numpy as np

B, P, NN, R, D = 128, 20, 32, 36, 1024
MARGIN = 0.2
M = 8            # cores
BC = B // M      # batches per core
NROW = P * NN    # 640 negative rows per batch
NPR = NROW + P   # 660 = negatives || positives moving rows
NPAD = 672       # padded so the DoubleRow rhs plane step is 16B-aligned
RPAD = 48        # img cols padded likewise for the stationary operand
DC = D // 128    # 8 chunks of the contraction dim
CP = DC // 2     # 4 DoubleRow chunk-pairs
HALF = NROW // 2  # 320, psum-bank sized neg-sims column split
GB = 4           # batches per SBUF tile ring slot (DMAs are per batch)
IMA_SPLIT = 4    # img batches loaded in the first small DMA
SW = BC * P      # 320 stacked (b, p) columns
CHUNK = 120      # stacked-phase transpose chunk (<=128 partitions, 6 batches)
NWARM = 50       # PE warmup matmuls (~6us busy: spans a full HAM window)

_compiled = None


def _build_program():
    from contextlib import ExitStack

    import concourse.tile as tile
    from concourse import bacc, mybir
    from concourse.masks import make_identity

    f32 = mybir.dt.float32
    f8 = mybir.dt.float8e4
    DR = mybir.MatmulPerfMode.DoubleRow
    AX = mybir.AxisListType.X
    AF = mybir.ActivationFunctionType
    OP = mybir.AluOpType

    nc = bacc.Bacc("TRN2", target_bir_lowering=False, debug=False, num_devices=M)

    npT = nc.dram_tensor("npT", [128, BC, DC, NPAD], f8, kind="ExternalInput").ap()
    imT = nc.dram_tensor("imT", [128, BC, DC, RPAD], f8, kind="ExternalInput").ap()

    # stacked chunks: two 120-wide slabs then two 40-wide so the serial tail
    # after the last batch only processes 2 batches' worth of columns
    chunks = [(0, CHUNK), (CHUNK, CHUNK), (2 * CHUNK, 40), (2 * CHUNK + 40, 40)]
    assert sum(w for _, w in chunks) == SW
    out = nc.dram_tensor("partial", [1, 1], f32, kind="ExternalOutput").ap()

    with tile.TileContext(nc) as tc, ExitStack() as ctx:
        singles = ctx.enter_context(tc.tile_pool(name="singles", bufs=1))
        nbuf = ctx.enter_context(tc.tile_pool(name="nbuf", bufs=3))
        small = ctx.enter_context(tc.tile_pool(name="small", bufs=2))
        ps_n0 = ctx.enter_context(tc.tile_pool(name="ps_n0", bufs=2, space="PSUM"))
        ps_n1 = ctx.enter_context(tc.tile_pool(name="ps_n1", bufs=2, space="PSUM"))
        ps_tr = ctx.enter_context(tc.tile_pool(name="ps_tr", bufs=2, space="PSUM"))

        # img load split in two so only batches 0-3's slice gates the first
        # matmul; the rest transfers behind the first negpos groups
        imaA = singles.tile([128, IMA_SPLIT, DC, RPAD], f8)
        imaB = singles.tile([128, BC - IMA_SPLIT, DC, RPAD], f8)
        garb = singles.tile([R, 256], f32)
        nc.vector.memset(garb, 1.0)
        nc.sync.dma_start(out=imaA, in_=imT[:, 0:IMA_SPLIT])

        ident = singles.tile([R, R], f32)
        make_identity(nc, ident)
        ones = singles.tile([CHUNK, 1], f32)
        nc.vector.memset(ones, 1.0)
        racc = singles.tile([CHUNK, len(chunks)], f32)
        nc.vector.memset(racc, 0.0)
        mxall = singles.tile([R, SW], f32)
        simsTall = singles.tile([R, SW], f32)

        # HAM warmup: dead fp32 matmuls keep the PE busy while the first
        # negpos group lands, so the real stream starts at 2.4GHz
        wps = ps_tr.tile([CHUNK, R], f32, tag="tr")
        for k in range(NWARM):
            nc.tensor.matmul(
                wps[0:R], ident, ident, start=(k == 0), stop=(k == NWARM - 1)
            )

        def stacked_chunk(k):
            c0, w = chunks[k]
            mxT_ps = ps_tr.tile([CHUNK, R], f32, tag="tr")
            nc.tensor.transpose(mxT_ps[0:w], mxall[:, c0 : c0 + w], ident)
            mxT = small.tile([CHUNK, R], f32, tag="mxT")
            nc.vector.tensor_copy(mxT[0:w], mxT_ps[0:w])

            smT_ps = ps_tr.tile([CHUNK, R], f32, tag="tr")
            nc.tensor.transpose(smT_ps[0:w], simsTall[:, c0 : c0 + w], ident)
            sims = small.tile([CHUNK, R], f32, tag="sims")
            nc.scalar.copy(sims[0:w], smT_ps[0:w])

            smax = small.tile([CHUNK, 1], f32, tag="smax")
            nc.vector.reduce_max(out=smax[0:w], in_=sims[0:w], axis=AX)
            nsmax = small.tile([CHUNK, 1], f32, tag="nsmax")
            nc.scalar.mul(nsmax[0:w], smax[0:w], -1.0)

            # E = exp(s - smax), den = sum_r E   (one ScalarE op)
            E = small.tile([CHUNK, R], f32, tag="E")
            den = small.tile([CHUNK, 1], f32, tag="den")
            nc.scalar.activation(
                E[0:w], sims[0:w], AF.Exp, bias=nsmax[0:w], scale=1.0,
                accum_out=den[0:w],
            )
            # hinge = relu(margin + maxneg - sims)
            hr = small.tile([CHUNK, R], f32, tag="hr")
            nc.vector.scalar_tensor_tensor(
                out=hr[0:w], in0=mxT[0:w], scalar=MARGIN, in1=sims[0:w],
                op0=OP.add, op1=OP.subtract,
            )
            h = small.tile([CHUNK, R], f32, tag="h")
            nc.scalar.activation(h[0:w], hr[0:w], AF.Relu)
            # W = E * h, num = sum_r W, ratio = num / den
            w_ = small.tile([CHUNK, R], f32, tag="w_")
            num = small.tile([CHUNK, 1], f32, tag="num")
            nc.vector.tensor_mul(w_[0:w], E[0:w], h[0:w])
            nc.vector.reduce_sum(out=num[0:w], in_=w_[0:w], axis=AX)
            rden = small.tile([CHUNK, 1], f32, tag="rden")
            nc.vector.reciprocal(rden[0:w], den[0:w])
            nc.vector.tensor_mul(racc[0:w, k : k + 1], num[0:w], rden[0:w])

        done_chunks = 0
        for ti in range(BC // GB):
            nt = nbuf.tile([128, GB, DC, NPAD], f8, tag="nt")
            # one dma_start per batch: completion semaphores fire per 0.69MB
            # slice, so the PE is never gated on a whole 2.8MB group landing
            for i in range(GB):
                nc.sync.dma_start(
                    out=nt[:, i : i + 1], in_=npT[:, ti * GB + i : ti * GB + i + 1]
                )
            if ti == 0:
                # rest of img, queued behind the first four negpos batches
                nc.sync.dma_start(out=imaB, in_=imT[:, IMA_SPLIT:BC])

            for i in range(GB):
                b = ti * GB + i
                ima, ib = (imaA, b) if b < IMA_SPLIT else (imaB, b - IMA_SPLIT)
                # one DoubleRow matmul stream per batch: [36 x 672] =
                # img.T @ (neg||pos||pad), split 320/352 across two psum banks
                ns0 = ps_n0.tile([R, HALF], f32, tag="ns0")
                ns1 = ps_n1.tile([R, NPAD - HALF], f32, tag="ns1")
                # dead fp32 matmul: HAM filler so the PE duty cycle stays high
                # enough to hold the 2.4GHz warm clock through the DR stream
                nc.tensor.matmul(
                    ns0[:, 0:256], ident, garb, start=True, stop=True
                )
                for c in range(CP):
                    st, sp = (c == 0), (c == CP - 1)
                    nc.tensor.matmul(
                        ns0, ima[:, ib, 2 * c : 2 * c + 2, 0:R],
                        nt[:, i, 2 * c : 2 * c + 2, 0:HALF],
                        start=st, stop=sp, perf_mode=DR,
                    )
                for c in range(CP):
                    st, sp = (c == 0), (c == CP - 1)
                    nc.tensor.matmul(
                        ns1, ima[:, ib, 2 * c : 2 * c + 2, 0:R],
                        nt[:, i, 2 * c : 2 * c + 2, HALF:NPAD],
                        start=st, stop=sp, perf_mode=DR,
                    )

                # max over the 32 negatives of each p -> mxall[:, 20b:20b+20];
                # transposed pos sims -> simsTall[:, 20b:20b+20]
                nc.vector.reduce_max(
                    out=mxall[:, P * b : P * b + P // 2],
                    in_=ns0.rearrange("r (g n) -> r g n", n=NN),
                    axis=AX,
                )
                nc.vector.reduce_max(
                    out=mxall[:, P * b + P // 2 : P * b + P],
                    in_=ns1[:, 0:HALF].rearrange("r (g n) -> r g n", n=NN),
                    axis=AX,
                )
                nc.scalar.copy(
                    simsTall[:, P * b : P * b + P], ns1[:, HALF : HALF + P]
                )

                # run a stacked chunk one batch after its sources complete, so
                # the in-order PE queue never waits on the DVE reduce deps
                while (
                    done_chunks < len(chunks)
                    and (b + 1) * P
                    >= chunks[done_chunks][0] + chunks[done_chunks][1] + P
                ):
                    stacked_chunk(done_chunks)
                    done_chunks += 1

        while done_chunks < len(chunks):
            stacked_chunk(done_chunks)
            done_chunks += 1

        # sum over chunks, then over the 120 partitions with a ones-matmul
        total = small.tile([CHUNK, 1], f32, tag="total")
        nc.vector.reduce_sum(out=total, in_=racc, axis=AX)
        fs = ps_tr.tile([1, 1], f32, tag="fs")
        nc.tensor.matmul(fs, total, ones, start=True, stop=True)
        res = small.tile([1, 1], f32, tag="res")
        nc.vector.tensor_copy(res, fs)
        nc.sync.dma_start(out=out, in_=res)

    nc.compile()
    return nc


def _maybe_trace_kwargs():
    """Optional NTFF profiling, enabled via BASS_LRL_TRACE=1 (used by test.py)."""
    if os.environ.get("BASS_LRL_TRACE") != "1":
        return {}
    import contextlib
    import ctypes
    import sys
    import types

    try:
        from antenv.axon_hooks import get_axon_ntff_profile_hook  # noqa: F401
    except ImportError:
        so_path = "/opt/axon/libaxon_pjrt.so"
        lib = ctypes.CDLL(so_path)
        lib.axon_start_nrt_profile.argtypes = [
            ctypes.POINTER(ctypes.c_int64),
            ctypes.c_size_t,
        ]
        lib.axon_start_nrt_profile.restype = ctypes.c_int64
        lib.axon_stop_nrt_profile.argtypes = [ctypes.c_char_p]
        lib.axon_stop_nrt_profile.restype = ctypes.c_int64

        @contextlib.contextmanager
        def _hook(output_dir, device_ids):
            import jax

            jax.devices()
            if device_ids:
                ids = (ctypes.c_int64 * len(device_ids))(*device_ids)
                rc = lib.axon_start_nrt_profile(ids, len(device_ids))
            else:
                rc = lib.axon_start_nrt_profile(None, 0)
            if rc != 0:
                raise RuntimeError(f"axon_start_nrt_profile rc={rc}")
            try:
                yield
            finally:
                n = lib.axon_stop_nrt_profile(str(output_dir).encode())
                if n <= 0:
                    print(f"WARNING: ntff capture wrote {n} files")

        mod = types.ModuleType("antenv.axon_hooks")
        mod.get_axon_ntff_profile_hook = lambda: _hook
        mod.set_axon_ntff_profile_hook = lambda h: None
        sys.modules["antenv.axon_hooks"] = mod

    import concourse.bass_utils as bu

    bu.upload_artifacts = lambda tmpdir: "local://" + tmpdir

    tmpdir = os.environ.get("BASS_LRL_TRACE_DIR", "/root/problem/trace_out")
    import shutil

    shutil.rmtree(tmpdir, ignore_errors=True)
    os.makedirs(tmpdir, exist_ok=True)
    kw = {"trace": True, "tmpdir": tmpdir}
    if os.environ.get("BASS_LRL_TRACE_ALL_CORES") == "1":
        kw["trace_cores"] = list(range(M))
    return kw


def _prep_inputs(img_feats, positives, negatives):
    """Build the per-core fp8-e3m4, D-major, partition-contiguous inputs."""
    import ml_dtypes

    f8 = ml_dtypes.float8_e4m3

    # negpos rows per batch: 640 negatives, 20 positives, 12 zero pad, D-major
    np_rows = np.zeros((B, NPAD, D), dtype=f8)
    np_rows[:, 0:NROW] = negatives.reshape(B, NROW, D).astype(f8)
    np_rows[:, NROW:NPR] = positives.astype(f8)
    # npT layout [128, BC, DC, NPAD]: [p, b, c, j] = rows[b, j, c*128+p]
    npt = np_rows.reshape(B, NPAD, DC, 128).transpose(3, 0, 2, 1)  # [p, b, c, j]

    img8 = np.zeros((B, RPAD, D), dtype=f8)
    img8[:, 0:R] = img_feats.astype(f8)
    # imT layout [128, BC, DC, RPAD]: [p, b, c, r] = img[b, r, c*128+p]
    imt = img8.reshape(B, RPAD, DC, 128).transpose(3, 0, 2, 1)  # [p, b, c, r] view

    in_maps = []
    for m in range(M):
        sl = slice(m * BC, (m + 1) * BC)
        in_maps.append(
            {
                "npT": np.ascontiguousarray(npt[:, sl]),
                "imT": np.ascontiguousarray(imt[:, sl]),
            }
        )
    return in_maps


def kernel(img_feats, positives, negatives):
    global _compiled
    from concourse.bass_utils import run_bass_kernel_spmd

    img_feats = np.asarray(img_feats, dtype=np.float32)
    positives = np.asarray(positives, dtype=np.float32)
    negatives = np.asarray(negatives, dtype=np.float32)
    assert img_feats.shape == (B, R, D)
    assert positives.shape == (B, P, D)
    assert negatives.shape == (B, P, NN, D)

    in_maps = _prep_inputs(img_feats, positives, negatives)

    if _compiled is None:
        _compiled = _build_program()
    nc = _compiled

    res = run_bass_kernel_spmd(nc, in_maps, list(range(M)), **_maybe_trace_kwargs())
    if res.exec_time_ns is not None:
        kernel.last_exec_time_ns = res.exec_time_ns
    partials = [np.float64(res.results[c]["partial"][0, 0]) for c in range(M)]
    return np.float32(sum(partials))


kernel.last_exec_time_ns = None


# revision 24
# speedup vs baseline: 1.2588x; 1.2588x over previous
"""Trainium2 Bass kernel for nn_LocalRegionLevelLoss (8-core data parallel).

loss = sum_{b,p,r} softmax_r(pos@img^T) * relu(margin + max_n(neg@img^T) - pos@img^T)

Strategy:
  - Pure data parallelism: batch dim (128) sharded 16-per-core across 8 cores.
  - fp8 E3M4 (float8e3) inputs: the kernel is HBM-bandwidth bound
    (45.6MB/core at fp32 vs a ~358GB/s/core HBM limit), and the loss is a
    2560-term softmax-weighted sum, so fp8 input quantization noise largely
    averages out (~7e-4 end-to-end rel err; e4m3 measured 1.3e-2).
  - The 20 positive rows are appended to the 640 negative rows of each
    batch's moving operand, so ONE stream of matmuls per batch produces
    both neg sims and (transposed) pos sims from the same stationary img
    chunks: no separate pos matmuls or extra weight loads.
  - HWDGE DMAs from one engine drain FIFO, so the negpos stream is issued
    in small-first groups (1,1,2,4,4,4 batches) to cut the pipeline-fill
    stall, while a burst of dummy ident matmuls pre-warms the PE HAM clock
    gate (cold PE = 1.2GHz, warm = 2.4GHz) during the first DMA.
  - Per batch only 3 cheap ops besides the matmuls: 2x reduce_max of the
    neg sims into a [36, 16*20] stack, 1 ScalarE copy of the transposed
    pos sims into a [36, 16*20] stack.  The softmax-weighted hinge runs 3x
    on [120, 36] PE-transposed chunks, interleaved into the batch loop as
    their source batches complete, so almost no serial tail remains.
  - Each core emits one f32 partial loss; host sums the 8 partials.
"""

import os
import numpy as np

B, P, NN, R, D = 128, 20, 32, 36, 1024
MARGIN = 0.2
M = 8            # cores
BC = B // M      # batches per core
NROW = P * NN    # 640 negative rows per batch
NPR = NROW + P   # 660 = negatives || positives moving rows
DC = D // 128    # 8 chunks of the contraction dim
HALF = NROW // 2  # 320, psum-bank sized neg-sims column split
GB = 4           # batches per SBUF tile ring slot (DMAs are per batch)
IMA_SPLIT = 4    # img batches loaded in the first small DMA
SW = BC * P      # 320 stacked (b, p) columns
CHUNK = 120      # stacked-phase transpose chunk (<=128 partitions, 6 batches)
NWARM = 32       # PE warmup matmuls (~3.8us busy, covers the HAM window)

_compiled = None


def _build_program():
    from contextlib import ExitStack

    import concourse.tile as tile
    from concourse import bacc, mybir
    from concourse.masks import make_identity

    f32 = mybir.dt.float32
    f8 = mybir.dt.float8e3
    AX = mybir.AxisListType.X
    AF = mybir.ActivationFunctionType
    OP = mybir.AluOpType

    nc = bacc.Bacc("TRN2", target_bir_lowering=False, debug=False, num_devices=M)

    npT = nc.dram_tensor("npT", [128, BC, DC, NPR], f8, kind="ExternalInput").ap()
    imT = nc.dram_tensor("imT", [128, BC, DC, R], f8, kind="ExternalInput").ap()

    # stacked chunks: two 120-wide slabs then two 40-wide so the serial tail
    # after the last batch only processes 2 batches' worth of columns
    chunks = [(0, CHUNK), (CHUNK, CHUNK), (2 * CHUNK, 40), (2 * CHUNK + 40, 40)]
    assert sum(w for _, w in chunks) == SW
    out = nc.dram_tensor("partial", [1, 1], f32, kind="ExternalOutput").ap()

    with tile.TileContext(nc) as tc, ExitStack() as ctx:
        singles = ctx.enter_context(tc.tile_pool(name="singles", bufs=1))
        nbuf = ctx.enter_context(tc.tile_pool(name="nbuf", bufs=3))
        small = ctx.enter_context(tc.tile_pool(name="small", bufs=2))
        ps_n0 = ctx.enter_context(tc.tile_pool(name="ps_n0", bufs=2, space="PSUM"))
        ps_n1 = ctx.enter_context(tc.tile_pool(name="ps_n1", bufs=2, space="PSUM"))
        ps_tr = ctx.enter_context(tc.tile_pool(name="ps_tr", bufs=2, space="PSUM"))

        # img load split in two so only batches 0-3's slice gates the first
        # matmul; the rest transfers behind the first negpos groups
        imaA = singles.tile([128, IMA_SPLIT, DC, R], f8)
        imaB = singles.tile([128, BC - IMA_SPLIT, DC, R], f8)
        nc.sync.dma_start(out=imaA, in_=imT[:, 0:IMA_SPLIT])

        ident = singles.tile([R, R], f32)
        make_identity(nc, ident)
        ones = singles.tile([CHUNK, 1], f32)
        nc.vector.memset(ones, 1.0)
        racc = singles.tile([CHUNK, len(chunks)], f32)
        nc.vector.memset(racc, 0.0)
        mxall = singles.tile([R, SW], f32)
        simsTall = singles.tile([R, SW], f32)

        # HAM warmup: dead fp32 matmuls keep the PE busy while the first
        # negpos group lands, so the real stream starts at 2.4GHz
        wps = ps_tr.tile([CHUNK, R], f32, tag="tr")
        for k in range(NWARM):
            nc.tensor.matmul(
                wps[0:R], ident, ident, start=(k == 0), stop=(k == NWARM - 1)
            )

        def stacked_chunk(k):
            c0, w = chunks[k]
            mxT_ps = ps_tr.tile([CHUNK, R], f32, tag="tr")
            nc.tensor.transpose(mxT_ps[0:w], mxall[:, c0 : c0 + w], ident)
            mxT = small.tile([CHUNK, R], f32, tag="mxT")
            nc.vector.tensor_copy(mxT[0:w], mxT_ps[0:w])

            smT_ps = ps_tr.tile([CHUNK, R], f32, tag="tr")
            nc.tensor.transpose(smT_ps[0:w], simsTall[:, c0 : c0 + w], ident)
            sims = small.tile([CHUNK, R], f32, tag="sims")
            nc.scalar.copy(sims[0:w], smT_ps[0:w])

            smax = small.tile([CHUNK, 1], f32, tag="smax")
            nc.vector.reduce_max(out=smax[0:w], in_=sims[0:w], axis=AX)
            nsmax = small.tile([CHUNK, 1], f32, tag="nsmax")
            nc.scalar.mul(nsmax[0:w], smax[0:w], -1.0)

            # E = exp(s - smax), den = sum_r E   (one ScalarE op)
            E = small.tile([CHUNK, R], f32, tag="E")
            den = small.tile([CHUNK, 1], f32, tag="den")
            nc.scalar.activation(
                E[0:w], sims[0:w], AF.Exp, bias=nsmax[0:w], scale=1.0,
                accum_out=den[0:w],
            )
            # hinge = relu(margin + maxneg - sims)
            hr = small.tile([CHUNK, R], f32, tag="hr")
            nc.vector.scalar_tensor_tensor(
                out=hr[0:w], in0=mxT[0:w], scalar=MARGIN, in1=sims[0:w],
                op0=OP.add, op1=OP.subtract,
            )
            h = small.tile([CHUNK, R], f32, tag="h")
            nc.scalar.activation(h[0:w], hr[0:w], AF.Relu)
            # W = E * h, num = sum_r W, ratio = num / den
            w_ = small.tile([CHUNK, R], f32, tag="w_")
            num = small.tile([CHUNK, 1], f32, tag="num")
            nc.vector.tensor_mul(w_[0:w], E[0:w], h[0:w])
            nc.vector.reduce_sum(out=num[0:w], in_=w_[0:w], axis=AX)
            rden = small.tile([CHUNK, 1], f32, tag="rden")
            nc.vector.reciprocal(rden[0:w], den[0:w])
            nc.vector.tensor_mul(racc[0:w, k : k + 1], num[0:w], rden[0:w])

        done_chunks = 0
        for ti in range(BC // GB):
            nt = nbuf.tile([128, GB, DC, NPR], f8, tag="nt")
            # one dma_start per batch: completion semaphores fire per 0.69MB
            # slice, so the PE is never gated on a whole 2.8MB group landing
            for i in range(GB):
                nc.sync.dma_start(
                    out=nt[:, i : i + 1], in_=npT[:, ti * GB + i : ti * GB + i + 1]
                )
            if ti == 0:
                # rest of img, queued behind the first four negpos batches
                nc.sync.dma_start(out=imaB, in_=imT[:, IMA_SPLIT:BC])

            for i in range(GB):
                b = ti * GB + i
                ima, ib = (imaA, b) if b < IMA_SPLIT else (imaB, b - IMA_SPLIT)
                # one matmul stream per batch: [36 x 660] = img.T @ (neg||pos),
                # split 320/340 across two psum banks
                ns0 = ps_n0.tile([R, HALF], f32, tag="ns0")
                ns1 = ps_n1.tile([R, NPR - HALF], f32, tag="ns1")
                for c in range(DC):
                    st, sp = (c == 0), (c == DC - 1)
                    nc.tensor.matmul(
                        ns0, ima[:, ib, c], nt[:, i, c, 0:HALF], start=st, stop=sp
                    )
                for c in range(DC):
                    st, sp = (c == 0), (c == DC - 1)
                    nc.tensor.matmul(
                        ns1, ima[:, ib, c], nt[:, i, c, HALF:NPR], start=st, stop=sp
                    )

                # max over the 32 negatives of each p -> mxall[:, 20b:20b+20];
                # transposed pos sims -> simsTall[:, 20b:20b+20]
                nc.vector.reduce_max(
                    out=mxall[:, P * b : P * b + P // 2],
                    in_=ns0.rearrange("r (g n) -> r g n", n=NN),
                    axis=AX,
                )
                nc.vector.reduce_max(
                    out=mxall[:, P * b + P // 2 : P * b + P],
                    in_=ns1[:, 0:HALF].rearrange("r (g n) -> r g n", n=NN),
                    axis=AX,
                )
                nc.scalar.copy(
                    simsTall[:, P * b : P * b + P], ns1[:, HALF : HALF + P]
                )

                # run a stacked chunk one batch after its sources complete, so
                # the in-order PE queue never waits on the DVE reduce deps
                while (
                    done_chunks < len(chunks)
                    and (b + 1) * P
                    >= chunks[done_chunks][0] + chunks[done_chunks][1] + P
                ):
                    stacked_chunk(done_chunks)
                    done_chunks += 1

        while done_chunks < len(chunks):
            stacked_chunk(done_chunks)
            done_chunks += 1

        # sum over chunks, then over the 120 partitions with a ones-matmul
        total = small.tile([CHUNK, 1], f32, tag="total")
        nc.vector.reduce_sum(out=total, in_=racc, axis=AX)
        fs = ps_tr.tile([1, 1], f32, tag="fs")
        nc.tensor.matmul(fs, total, ones, start=True, stop=True)
        res = small.tile([1, 1], f32, tag="res")
        nc.vector.tensor_copy(res, fs)
        nc.sync.dma_start(out=out, in_=res)

    nc.compile()
    return nc


def _maybe_trace_kwargs():
    """Optional NTFF profiling, enabled via BASS_LRL_TRACE=1 (used by test.py)."""
    if os.environ.get("BASS_LRL_TRACE") != "1":
        return {}
    import contextlib
    import ctypes
    import sys
    import types

    try:
        from antenv.axon_hooks import get_axon_ntff_profile_hook  # noqa: F401
    except ImportError:
        so_path = "/opt/axon/libaxon_pjrt.so"
        lib = ctypes.CDLL(so_path)
        lib.axon_start_nrt_profile.argtypes = [
            ctypes.POINTER(ctypes.c_int64),
            ctypes.c_size_t,
        ]
        lib.axon_start_nrt_profile.restype = ctypes.c_int64
        lib.axon_stop_nrt_profile.argtypes = [ctypes.c_char_p]
        lib.axon_stop_nrt_profile.restype = ctypes.c_int64

        @contextlib.contextmanager
        def _hook(output_dir, device_ids):
            import jax

            jax.devices()
            if device_ids:
                ids = (ctypes.c_int64 * len(device_ids))(*device_ids)
                rc = lib.axon_start_nrt_profile(ids, len(device_ids))
            else:
                rc = lib.axon_start_nrt_profile(None, 0)
            if rc != 0:
                raise RuntimeError(f"axon_start_nrt_profile rc={rc}")
            try:
                yield
            finally:
                n = lib.axon_stop_nrt_profile(str(output_dir).encode())
                if n <= 0:
                    print(f"WARNING: ntff capture wrote {n} files")

        mod = types.ModuleType("antenv.axon_hooks")
        mod.get_axon_ntff_profile_hook = lambda: _hook
        mod.set_axon_ntff_profile_hook = lambda h: None
        sys.modules["antenv.axon_hooks"] = mod

    import concourse.bass_utils as bu

    bu.upload_artifacts = lambda tmpdir: "local://" + tmpdir

    tmpdir = os.environ.get("BASS_LRL_TRACE_DIR", "/root/problem/trace_out")
    import shutil

    shutil.rmtree(tmpdir, ignore_errors=True)
    os.makedirs(tmpdir, exist_ok=True)
    kw = {"trace": True, "tmpdir": tmpdir}
    if os.environ.get("BASS_LRL_TRACE_ALL_CORES") == "1":
        kw["trace_cores"] = list(range(M))
    return kw


def _prep_inputs(img_feats, positives, negatives):
    """Build the per-core fp8-e3m4, D-major, partition-contiguous inputs."""
    import ml_dtypes

    f8 = ml_dtypes.float8_e3m4

    # negpos rows per batch: 640 negatives then 20 positives, D-major
    np_rows = np.concatenate(
        [negatives.reshape(B, NROW, D), positives], axis=1
    ).astype(f8)  # [B, 660, D]
    # npT layout [128, BC, DC, NPR]: [p, b, c, j] = rows[b, j, c*128+p]
    npt = np_rows.reshape(B, NPR, DC, 128).transpose(3, 0, 2, 1)  # [p, b, c, j] view

    img8 = img_feats.astype(f8)
    # imT layout [128, BC, DC, R]: [p, b, c, r] = img[b, r, c*128+p]
    imt = img8.reshape(B, R, DC, 128).transpose(3, 0, 2, 1)  # [p, b, c, r] view

    in_maps = []
    for m in range(M):
        sl = slice(m * BC, (m + 1) * BC)
        in_maps.append(
            {
                "npT": np.ascontiguousarray(npt[:, sl]),
                "imT": np.ascontiguousarray(imt[:, sl]),
            }
        )
    return in_maps


def kernel(img_feats, positives, negatives):
    global _compiled
    from concourse.bass_utils import run_bass_kernel_spmd

    img_feats = np.asarray(img_feats, dtype=np.float32)
    positives = np.asarray(positives, dtype=np.float32)
    negatives = np.asarray(negatives, dtype=np.float32)
    assert img_feats.shape == (B, R, D)
    assert positives.shape == (B, P, D)
    assert negatives.shape == (B, P, NN, D)

    in_maps = _prep_inputs(img_feats, positives, negatives)

    if _compiled is None:
        _compiled = _build_program()
    nc = _compiled

    res = run_bass_kernel_spmd(nc, in_maps, list(range(M)), **_maybe_trace_kwargs())
    if res.exec_time_ns is not None:
        kernel.last_exec_time_ns = res.exec_time_ns
    partials = [np.float64(res.results[c]["partial"][0, 0]) for c in range(M)]
    return np.float32(sum(partials))


kernel.last_exec_time_ns = None
